# revision 1
# baseline (speedup 1.0000x reference)
"""CNF block (RK4 ODE-int + Hutchinson divergence) Trainium2 Bass kernel.

Data-parallel over the flattened N = seq*bsz*ns = 32768 row dimension across
8 NeuronCores (4096 rows/core). Feature-major layout on device ([256 feats on
2x128 partitions, rows on the free dim]).

Algorithm (validated vs reference in fp64/bf16 numpy model):
  pre1_1 = A1@z + c1(t)                     (G, kept for replay)
  pre1_i = G + a_{i-1}*B@h1_{i-1} + c1eff   (B = A1@A2 host-precomputed)
  E = Exp(pre1 + bias); h1 = Ln(E+1) [= softplus, exact]
  s' = Exp(-h1) [= 1/(1+E)], sigma = 1 - s'
  div_i = sum_j sigma*w = sumw(host) - colsum(s' (.) w)
  z_{s+1} = z + (dt/6)A2@(h1_1+h1_4) + (dt/3)A2@(h1_2+h1_3) + zbias(host)
  delta = -sum c_i div_i ; out = log_pz0(host) - delta
All t-dependent bias terms folded into per-stage per-partition ACT bias APs.
"""
import math
import sys

import numpy as np

try:
    import concourse.bacc as bacc
except ImportError:
    sys.path.insert(0, "/opt/trn_rl_repo")
    import concourse.bacc as bacc

import concourse.tile as tile
from concourse import mybir
from concourse.bass_utils import run_bass_kernel_spmd

try:
    import ml_dtypes

    _BF16 = ml_dtypes.bfloat16
except ImportError:  # pragma: no cover
    _BF16 = None

N_CORES = 8
D = 256
SEQ, BSZ, NS = 32, 16, 64
N_TOTAL = SEQ * BSZ * NS            # 32768
N_PER = N_TOTAL // N_CORES          # 4096
NSTEPS = 8
DT = 1.0 / NSTEPS
GRAN = 1024                         # psum granule columns
NGRAN = N_PER // GRAN               # 4
NSPAN = N_PER // 512                # 8 (div-reduce spans)
DBG_ZOUT = False

_dt_bf = mybir.dt.bfloat16
_dt_f32 = mybir.dt.float32

_cache = {}


def _build_program():
    nc = bacc.Bacc("TRN2", target_bir_lowering=False, debug=False,
                   num_devices=N_CORES)

    d_zT = nc.dram_tensor("zT", [2, 128, N_PER], _dt_bf, kind="ExternalInput").ap()
    d_wT = nc.dram_tensor("wT", [2, 128, N_PER], _dt_bf, kind="ExternalInput").ap()
    d_lA1 = nc.dram_tensor("lA1", [256, 256], _dt_bf, kind="ExternalInput").ap()
    d_lB1 = nc.dram_tensor("lB1", [256, 256], _dt_bf, kind="ExternalInput").ap()
    d_lB2 = nc.dram_tensor("lB2", [256, 256], _dt_bf, kind="ExternalInput").ap()
    d_lG1 = nc.dram_tensor("lG1", [256, 256], _dt_bf, kind="ExternalInput").ap()
    d_lG2 = nc.dram_tensor("lG2", [256, 256], _dt_bf, kind="ExternalInput").ap()
    d_ident = nc.dram_tensor("ident", [128, 128], _dt_bf, kind="ExternalInput").ap()
    d_ones = nc.dram_tensor("onesv", [128, 2], _dt_bf, kind="ExternalInput").ap()
    d_biasP = nc.dram_tensor("biasP", [128, 64], _dt_f32, kind="ExternalInput").ap()
    d_zbias = nc.dram_tensor("zbias", [128, 16], _dt_f32, kind="ExternalInput").ap()

    d_div = nc.dram_tensor("divout", [4, GRAN], _dt_f32, kind="ExternalOutput").ap()
    if DBG_ZOUT:
        d_zout = nc.dram_tensor("zout", [2, 128, N_PER], _dt_bf,
                                kind="ExternalOutput").ap()

    from contextlib import ExitStack

    with tile.TileContext(nc) as tc:
        with ExitStack() as ctx:
            consts = ctx.enter_context(tc.tile_pool(name="consts", bufs=1))
            sb = ctx.enter_context(tc.tile_pool(name="sb", bufs=1))
            ps_pool = ctx.enter_context(tc.tile_pool(name="ps", bufs=3, space="PSUM"))
            div_pool = ctx.enter_context(tc.tile_pool(name="divp", bufs=1, space="PSUM"))

            # ---- weights packed [128, 512]: col block (k2*2+j2)*128 ----
            def load_w(name, dram):
                t = consts.tile([128, 512], _dt_bf, name=name)
                for k2 in range(2):
                    for j2 in range(2):
                        c = (k2 * 2 + j2) * 128
                        nc.sync.dma_start(t[:, c:c + 128],
                                          dram[k2 * 128:(k2 + 1) * 128,
                                               j2 * 128:(j2 + 1) * 128])
                return t

            wA1 = load_w("wA1", d_lA1)
            wB1 = load_w("wB1", d_lB1)
            wB2 = load_w("wB2", d_lB2)
            wG1 = load_w("wG1", d_lG1)
            wG2 = load_w("wG2", d_lG2)

            ident = consts.tile([128, 128], _dt_bf, name="ident")
            nc.sync.dma_start(ident[:], d_ident[:, :])
            onesv = consts.tile([128, 2], _dt_bf, name="onesv")
            nc.sync.dma_start(onesv[:], d_ones[:, :])
            biasP = consts.tile([128, 64], _dt_f32, name="biasP")
            nc.sync.dma_start(biasP[:], d_biasP[:, :])
            zbias = consts.tile([128, 16], _dt_f32, name="zbias")
            nc.sync.dma_start(zbias[:], d_zbias[:, :])

            def fw(name):  # full-width bf16 tile pair (one per ftile)
                return [sb.tile([128, N_PER], _dt_bf, name=f"{name}{f}")
                        for f in range(2)]

            zT = fw("zT")
            wT = fw("wT")
            for f in range(2):
                nc.sync.dma_start(zT[f][:], d_zT[f])
                nc.sync.dma_start(wT[f][:], d_wT[f])
            Ebuf = fw("Ebuf")
            Gbuf = fw("Gbuf")
            hA = fw("hA")
            hB = fw("hB")
            hC = fw("hC")
            hD = fw("hD")
            spb = fw("spb")
            qb = fw("qb")
            g1b = fw("g1b")
            g2b = fw("g2b")
            hbufs = [hA, hB, hC, hD]

            divps = div_pool.tile([128, 1024], _dt_f32, name="divps")

            Exp = mybir.ActivationFunctionType.Exp
            Ln = mybir.ActivationFunctionType.Ln
            add_op = mybir.AluOpType.add

            for s in range(NSTEPS):
                for i in range(4):
                    hprev = hbufs[i - 1] if i > 0 else None
                    lB = wB1 if i < 3 else wB2
                    bcol = s * 4 + i
                    for f in range(2):
                        for g in range(NGRAN):
                            g0 = g * GRAN
                            ps = ps_pool.tile([128, GRAN], _dt_f32, tag="pre1",
                                              name=f"ps_{s}_{i}_{f}_{g}")
                            if i == 0:
                                for k2 in range(2):
                                    lhs = wA1[:, (k2 * 2 + f) * 128:(k2 * 2 + f) * 128 + 128]
                                    for nn in range(2):
                                        cs = slice(nn * 512, (nn + 1) * 512)
                                        nc.tensor.matmul(
                                            ps[:, cs], lhs,
                                            zT[k2][:, g0 + nn * 512:g0 + (nn + 1) * 512],
                                            start=(k2 == 0), stop=(k2 == 1))
                            else:
                                for nn in range(2):
                                    cs = slice(nn * 512, (nn + 1) * 512)
                                    nc.tensor.matmul(
                                        ps[:, cs], ident[:],
                                        Gbuf[f][:, g0 + nn * 512:g0 + (nn + 1) * 512],
                                        start=True, stop=False)
                                for k2 in range(2):
                                    lhs = lB[:, (k2 * 2 + f) * 128:(k2 * 2 + f) * 128 + 128]
                                    for nn in range(2):
                                        cs = slice(nn * 512, (nn + 1) * 512)
                                        nc.tensor.matmul(
                                            ps[:, cs], lhs,
                                            hprev[k2][:, g0 + nn * 512:g0 + (nn + 1) * 512],
                                            start=False, stop=(k2 == 1))
                            # E = Exp(pre1 + bias)
                            nc.scalar.activation(
                                Ebuf[f][:, g0:g0 + GRAN], ps[:], Exp,
                                bias=biasP[:, f * 32 + bcol:f * 32 + bcol + 1],
                                scale=1.0)
                            if i == 0:
                                # keep raw G = A1@z for replays
                                nc.vector.tensor_copy(Gbuf[f][:, g0:g0 + GRAN], ps[:])
                        # full-width: h1 = Ln(E+1); s' = Exp(-h1)
                        h1 = hbufs[i][f]
                        nc.scalar.activation(h1[:], Ebuf[f][:], Ln, bias=1.0, scale=1.0)
                        nc.scalar.activation(spb[f][:], h1[:], Exp, bias=0.0, scale=-1.0)
                        # q' = s' * w ; div-reduce with chat in ones column
                        nc.vector.tensor_mul(qb[f][:], spb[f][:], wT[f][:])
                        onecol = 0 if i in (0, 3) else 1
                        for sp in range(NSPAN):
                            p = 32 * (sp % 4)
                            ch = 512 * (sp // 4)
                            first = (s == 0 and i == 0 and f == 0)
                            last = (s == NSTEPS - 1 and i == 3 and f == 1)
                            nc.tensor.matmul(
                                divps[p:p + 1, ch:ch + 512],
                                onesv[:, onecol:onecol + 1],
                                qb[f][:, sp * 512:sp * 512 + 512],
                                start=first, stop=last,
                                tile_position=(0, p), skip_group_check=True)
                    if i == 2:
                        for f in range(2):
                            nc.vector.tensor_add(g2b[f][:], hB[f][:], hC[f][:])
                # ---- step end: g1, z update ----
                for f in range(2):
                    nc.vector.tensor_add(g1b[f][:], hA[f][:], hD[f][:])
                for f in range(2):
                    for g in range(NGRAN):
                        g0 = g * GRAN
                        zu = ps_pool.tile([128, GRAN], _dt_f32, tag="pre1",
                                          name=f"zu_{s}_{f}_{g}")
                        for nn in range(2):
                            cs = slice(nn * 512, (nn + 1) * 512)
                            nc.tensor.matmul(
                                zu[:, cs], ident[:],
                                zT[f][:, g0 + nn * 512:g0 + (nn + 1) * 512],
                                start=True, stop=False)
                        for wg, gb in ((wG1, g1b), (wG2, g2b)):
                            for k2 in range(2):
                                lhs = wg[:, (k2 * 2 + f) * 128:(k2 * 2 + f) * 128 + 128]
                                for nn in range(2):
                                    cs = slice(nn * 512, (nn + 1) * 512)
                                    nc.tensor.matmul(
                                        zu[:, cs], lhs,
                                        gb[k2][:, g0 + nn * 512:g0 + (nn + 1) * 512],
                                        start=False,
                                        stop=(wg is wG2 and k2 == 1))
                        nc.vector.tensor_scalar(
                            zT[f][:, g0:g0 + GRAN], zu[:],
                            zbias[:, f * 8 + s:f * 8 + s + 1], None, add_op)

            # ---- drain div accumulator ----
            stag = sb.tile([128, GRAN], _dt_f32, name="stag")
            for r in range(4):
                nc.vector.tensor_copy(stag[32 * r:32 * r + 1, :],
                                      divps[32 * r:32 * r + 1, :])
                nc.sync.dma_start(d_div[r:r + 1, :], stag[32 * r:32 * r + 1, :])
            if DBG_ZOUT:
                for f in range(2):
                    nc.sync.dma_start(d_zout[f], zT[f][:])

    nc.compile()
    return nc


def _host_prep(h, emb_matrix, W1, b1, W2, b2, sampled_targets):
    import jax

    cpu = jax.devices("cpu")[0]

    A1 = W1[:, 1:].astype(np.float64)
    w1t = W1[:, 0].astype(np.float64)
    A2 = W2[:, 1:].astype(np.float64)
    w2t = W2[:, 0].astype(np.float64)
    b1d = b1.astype(np.float64)
    b2d = b2.astype(np.float64)
    B = A1 @ A2
    A1c2w = A1 @ w2t
    A1c2b = A1 @ b2d

    idx = np.asarray(sampled_targets).reshape(-1).astype(np.int64)
    z0 = emb_matrix.astype(np.float64)[idx]                 # [N, d]
    hf = h.reshape(SEQ * BSZ, D).astype(np.float64)
    mu = np.repeat(hf, NS, axis=0)
    log_pz0 = (-0.5 * np.sum((z0 - mu) ** 2, axis=1)
               - 0.5 * D * math.log(2.0 * math.pi))

    with jax.default_device(cpu):
        e = np.asarray(jax.random.bernoulli(jax.random.key(42), 0.5,
                                            (N_TOTAL, D))).astype(np.float64) * 2.0 - 1.0
    u = e @ A1.T
    v = e @ A2
    w = u * v
    sumw = w.sum(axis=1)

    def c1(t):
        return t * w1t + b1d

    def c2(t):
        return t * w2t + b2d

    # per-(step, stage) ACT bias vectors and per-step z-bias vectors
    biasP = np.zeros((128, 64), np.float32)
    zbias = np.zeros((128, 16), np.float32)
    for s in range(NSTEPS):
        ts = s * DT
        taus = [ts, ts + DT / 2, ts + DT / 2, ts + DT]
        for i in range(4):
            if i == 0:
                bv = c1(taus[0])
            else:
                alpha = DT / 2 if i < 3 else DT
                bv = c1(taus[i]) + alpha * (taus[i - 1] * A1c2w + A1c2b)
            for f in range(2):
                biasP[:, f * 32 + s * 4 + i] = bv[f * 128:(f + 1) * 128]
        zb = (DT / 6) * (c2(taus[0]) + 2 * c2(taus[1]) + 2 * c2(taus[2])
                         + c2(taus[3]))
        for f in range(2):
            zbias[:, f * 8 + s] = zb[f * 128:(f + 1) * 128]

    def bf16(x):
        return np.asarray(x, np.float32).astype(_BF16)

    weights = {
        "lA1": bf16(A1.T),
        "lB1": bf16((DT / 2 * B).T),
        "lB2": bf16((DT * B).T),
        "lG1": bf16((DT / 6 * A2).T),
        "lG2": bf16((DT / 3 * A2).T),
        "ident": bf16(np.eye(128)),
        "onesv": bf16(np.stack([np.ones(128), 2.0 * np.ones(128)], axis=1)),
        "biasP": biasP,
        "zbias": zbias,
    }

    in_maps = []
    for c in range(N_CORES):
        rows = slice(c * N_PER, (c + 1) * N_PER)
        zTc = bf16(z0[rows].T.reshape(2, 128, N_PER))
        wTc = bf16(w[rows].T.reshape(2, 128, N_PER))
        in_maps.append({"zT": zTc, "wT": wTc, **weights})
    return in_maps, log_pz0, sumw


def kernel(h, emb_matrix, W1, b1, W2, b2, sampled_targets):
    in_maps, log_pz0, sumw = _host_prep(h, emb_matrix, W1, b1, W2, b2,
                                        sampled_targets)
    if "nc" not in _cache:
        _cache["nc"] = _build_program()
    nc = _cache["nc"]

    res = run_bass_kernel_spmd(nc, in_maps, list(range(N_CORES)))

    R = np.zeros(N_TOTAL, np.float64)
    for c in range(N_CORES):
        dv = res.results[c]["divout"].astype(np.float64)    # [4, 1024]
        for sp in range(NSPAN):
            r = sp % 4
            ch = 512 * (sp // 4)
            R[c * N_PER + sp * 512: c * N_PER + (sp + 1) * 512] = \
                dv[r, ch:ch + 512]
    delta = -(sumw - (DT / 6) * R)
    out = (log_pz0 - delta).reshape(SEQ * BSZ, NS)
    return out.astype(np.float32)


if __name__ == "__main__":
    pass
